# revision 9
# baseline (speedup 1.0000x reference)
"""AttentiveManifoldMixer Trainium2 kernel (8-core data parallel over batch).

Math: with W3[c,i,j] = conv_w[c*64+i, j], B = conv_b.reshape(C, C),
  s[b]       = sigmoid(fc2 @ relu(fc1 @ mean_hw(x[b])))
  out[b,c,p] = sum_{i,j} W3[c,i,j] * s[b,j] * x[b,i,p] * x[b,j,p]
               + sum_i B[c,i] * x[b,i,p]

The quadratic form is symmetrized over unordered channel pairs grouped by
cyclic diagonal offset d; 17 chunks x 128 lanes cover d = 0..35 (dup offsets
at higher mult).  Lane q = 2*i + h of chunk m = 3k + l holds
x_{(i-6k)%64} * x_{(i+2l+h)%64} (offset d = 6k+2l+h), built as one bf16
tensor_tensor of two DMA-loaded variant tiles:
  A_k[q] = x[(i-6k)%64]   (in1, shared by chunks 3k..3k+2)
  B_l[q] = x[(i+2l+h)%64] (in2, shared by chunks l, l+3, ...)
The parity-interleaved lane order makes every variant tile a SINGLE
128-partition full-rate DMA from a doubled bf16 copy of x in DRAM
(access pattern [[1,64],[0|1,2]] rows).  A_k loads go on the gpsimd
(SWDGE) queue, B_l/x/out on the sync queue, small traffic on scalar.

GEMM: per chunk one bf16 LDW+matmul pair per psum bank, column-tiled on
the PE: psum partitions 0:64 (tile (0,0)) and 64:128 (tile (0,64)) hold
two different 512-pixel blocks streaming concurrently through the two
column halves of the PE array -> ~2x matmul throughput at M=64.  Pixel
blocks are processed in two interleaved groups {0,1,4,5} / {2,3,6,7} so
each psum bank pairs blocks (j, j+4); a full [128,512] ACT copy then
assembles a [128,2048] staging tile and ONE full-rate DMA per group
writes out.  conv_b is a bf16 identity-init matmul folded into the same
accumulation.  The whole dataflow is pipelined over the two column
groups so the first group's compute overlaps the second's DMA stream.
"""
import sys

sys.path.insert(0, "/opt/trn_rl_repo")

import numpy as np
import ml_dtypes

B, C, H, W = 8, 64, 64, 64
P = H * W                  # 4096 pixels per sample
MID = C // 4
NCHUNK = 17                # feature chunks
NA, NB = 6, 3              # A/B variant tiles; chunk m = 3*(m//3) + m%3
NSUB = 512                 # matmul free-dim subtile / psum bank cols
NS = P // NSUB             # pixel blocks
N_CORES = 8
HALF = P // 2              # cols per pipeline group (2048)

_CACHE = {}


def _lane_maps():
    """Per-lane channel pair: chunk m = 3k+l, lane q = 2*i + h:
    in1-index i1 = (i - 6k) % 64, in2-index j1 = (i + 2l + h) % 64."""
    i_idx = np.zeros((NCHUNK, 128), np.int64)
    j_idx = np.zeros((NCHUNK, 128), np.int64)
    for m in range(NCHUNK):
        k, l = divmod(m, 3)
        for q in range(128):
            i, h = divmod(q, 2)
            i_idx[m, q] = (i - 6 * k) % 64
            j_idx[m, q] = (i + 2 * l + h) % 64
    lo = np.minimum(i_idx, j_idx)
    hi = np.maximum(i_idx, j_idx)
    key = lo * 64 + hi
    _, inv, counts = np.unique(key, return_inverse=True, return_counts=True)
    mult = counts[inv].reshape(key.shape).astype(np.float32)
    return i_idx, j_idx, mult


def _host_weights(conv_w, fc1_w, fc2_w):
    """Pre-gather conv_w into per-lane arrays a1/a2 of shape (128, 17, 64):
    [lane q, chunk m, out-channel c], bf16."""
    w3 = conv_w.reshape(C, C, C)  # [c, i, j]
    i_idx, j_idx, mult = _lane_maps()
    a1 = np.transpose(w3[:, i_idx, j_idx], (2, 1, 0)) / mult.T[:, :, None]
    a2 = np.transpose(w3[:, j_idx, i_idx], (2, 1, 0)) / mult.T[:, :, None]
    diag = (i_idx == j_idx).T  # [q, m]
    a2[diag] = 0.0
    fc1t = (fc1_w.T / float(P)).copy()   # (64, 16): folds the 1/HW of the mean
    fc2t = fc2_w.T.copy()                # (16, 64)
    return (np.ascontiguousarray(a1, ml_dtypes.bfloat16),
            np.ascontiguousarray(a2, ml_dtypes.bfloat16), fc1t, fc2t)


def _build_program(niter=None):
    """Build the kernel program; with niter, wrap the body in an on-device
    For_i repeat loop (timing variant)."""
    import contextlib

    import concourse.bacc as bacc
    import concourse.bass as bass
    from concourse import mybir
    from concourse.tile import TileContext

    nc = bacc.Bacc("TRN2", target_bir_lowering=False, debug=False)
    dt = mybir.dt

    x_d = nc.dram_tensor("x", [C, P], dt.float32, kind="ExternalInput")
    a1_d = nc.dram_tensor("a1", [128, NCHUNK, C], dt.bfloat16, kind="ExternalInput")
    a2_d = nc.dram_tensor("a2", [128, NCHUNK, C], dt.bfloat16, kind="ExternalInput")
    f1_d = nc.dram_tensor("fc1t", [C, MID], dt.float32, kind="ExternalInput")
    f2_d = nc.dram_tensor("fc2t", [MID, C], dt.float32, kind="ExternalInput")
    id_d = nc.dram_tensor("identb", [C, C], dt.bfloat16, kind="ExternalInput")
    out_d = nc.dram_tensor("out", [C, P], dt.float32, kind="ExternalOutput")

    # pixel-group permutation: group g holds true pixel blocks {g*2, g*2+1,
    # g*2+4, g*2+5}... actually blocks {0,1,4,5} for g=0, {2,3,6,7} for g=1:
    # local block b in group g -> true block TB[g][b]
    TB = [[0, 1, 4, 5], [2, 3, 6, 7]]

    with TileContext(nc) as tc:
        with tc.tile_pool(name="single", bufs=1) as single, \
             tc.tile_pool(name="dram", bufs=1, space="DRAM") as dpool, \
             tc.tile_pool(name="feat", bufs=6) as featp, \
             tc.tile_pool(name="outs", bufs=2) as outsp, \
             tc.tile_pool(name="psum", bufs=8, space="PSUM") as psum, \
             (tc.For_i(0, niter, 1,
                       hint_engines=(mybir.EngineType.PE,
                                     mybir.EngineType.DVE,
                                     mybir.EngineType.SP,
                                     mybir.EngineType.Pool,
                                     mybir.EngineType.Activation))
              if niter else contextlib.nullcontext()):

            gsls = [slice(g * HALF, (g + 1) * HALF) for g in range(2)]

            # ---- load x (fp32): group g cols = true blocks TB[g] ----
            xf = single.tile([C, P], dt.float32)
            for g in range(2):
                # src runs: [g*1024 : g*1024+1024] and [2048+g*1024 : ...]
                src = bass.AP(tensor=x_d.ap().tensor,
                              offset=x_d.ap().offset + g * 1024,
                              ap=[[P, C], [2048, 2], [1, 1024]])
                eng = nc.sync if g == 0 else nc.gpsimd
                eng.dma_start(out=xf[:, gsls[g]], in_=src)

            # ---- small weight loads ----
            a1s = single.tile([128, NCHUNK, C], dt.bfloat16)
            nc.gpsimd.dma_start(out=a1s, in_=a1_d.ap())
            a2s = single.tile([128, NCHUNK, C], dt.bfloat16)
            nc.gpsimd.dma_start(out=a2s, in_=a2_d.ap())
            f1s = single.tile([C, MID], dt.float32)
            nc.gpsimd.dma_start(out=f1s, in_=f1_d.ap())
            f2s = single.tile([MID, C], dt.float32)
            nc.gpsimd.dma_start(out=f2s, in_=f2_d.ap())
            ids = single.tile([C, C], dt.bfloat16)
            nc.gpsimd.dma_start(out=ids, in_=id_d.ap())

            # ---- cast x -> bf16 (+ per-group channel sums), double rows in
            # DRAM, stream variant loads.
            xb = single.tile([C, P], dt.bfloat16)
            sums_h = [single.tile([C, 1], dt.float32, name=f"sums{g}")
                      for g in range(2)]
            xb2_dram = dpool.tile([128, P], dt.bfloat16)
            a_tiles = [single.tile([128, P], dt.bfloat16, name=f"av{k}")
                       for k in range(NA)]
            b_tiles = [single.tile([128, P], dt.bfloat16, name=f"bv{l}")
                       for l in range(NB)]

            def var_src(base_row, dup, gsl):
                """AP over xb2_dram rows: partition q=2i+h -> row base+i(+h)."""
                return bass.AP(
                    tensor=xb2_dram.tensor,
                    offset=xb2_dram.offset + base_row * P + gsl.start,
                    ap=[[P, 64], [0 if dup else P, 2],
                        [1, gsl.stop - gsl.start]])

            for g, gsl in enumerate(gsls):
                nc.scalar.activation(xb[:, gsl], xf[:, gsl],
                                     mybir.ActivationFunctionType.Copy,
                                     accum_out=sums_h[g])
                nc.sync.dma_start(out=xb2_dram[0:C, gsl], in_=xb[:, gsl])
                nc.sync.dma_start(out=xb2_dram[C:128, gsl], in_=xb[:, gsl])

                # A0 (k=0, rows 0..63 dup) + B tiles on sync queue per group
                nc.sync.dma_start(out=a_tiles[0][:, gsl],
                                  in_=var_src(0, True, gsl))
                for l in range(NB):
                    nc.sync.dma_start(out=b_tiles[l][:, gsl],
                                      in_=var_src(2 * l, False, gsl))
            # A1..A5: full-width single loads on the gpsimd (SWDGE) queue
            for k in range(1, NA):
                nc.gpsimd.dma_start(out=a_tiles[k],
                                    in_=var_src(64 - 6 * k, True, slice(0, P)))

            # ---- SE path: s = sigmoid(fc2t.T @ relu(fc1t.T @ sums)) ----
            ps1 = psum.tile([MID, 1], dt.float32, tag="acc")
            for g in range(2):
                nc.tensor.matmul(ps1, f1s, sums_h[g], start=(g == 0),
                                 stop=(g == 1))
            y1 = single.tile([MID, 1], dt.float32)
            nc.scalar.activation(y1, ps1, mybir.ActivationFunctionType.Relu)
            ps2 = psum.tile([C, 1], dt.float32, tag="acc")
            nc.tensor.matmul(ps2, f2s, y1, start=True, stop=True)
            svec = single.tile([C, 1], dt.float32)
            nc.scalar.activation(svec, ps2, mybir.ActivationFunctionType.Sigmoid)

            # s -> DRAM twice (s_int = [s; s]) for the gather DMAs
            s_int = dpool.tile([2 * C], dt.float32)
            nc.scalar.dma_start(out=s_int[0:C][:, None], in_=svec)
            nc.scalar.dma_start(out=s_int[C:2 * C][:, None], in_=svec)

            # gathers (lane q = 2i+h):
            # S1b[q, l] = s[j(l, q)] = s_int[i + 2l + h]  (3 cols)
            s1b = single.tile([128, NB], dt.float32)
            for l in range(NB):
                nc.scalar.dma_start(
                    out=s1b[:, l:l + 1],
                    in_=bass.AP(tensor=s_int.tensor,
                                offset=s_int.offset + 2 * l,
                                ap=[[1, 64], [1, 2], [0, 1]]))
            # S2b[q, k] = s[i(k, q)] = s_int[i + 64 - 6k]  (6 cols)
            s2b = single.tile([128, NA], dt.float32)
            for k in range(NA):
                nc.scalar.dma_start(
                    out=s2b[:, k:k + 1],
                    in_=bass.AP(tensor=s_int.tensor,
                                offset=s_int.offset + (64 - 6 * k) % 64,
                                ap=[[1, 64], [0, 2], [0, 1]]))

            # ---- fold s into weights: wc = a1*S1 + a2*S2 (bf16) ----
            wc = single.tile([128, NCHUNK, C], dt.bfloat16)
            t1 = single.tile([128, NCHUNK, C], dt.float32)
            t2 = single.tile([128, NCHUNK, C], dt.float32)
            for l in range(NB):
                nc.scalar.mul(t1[:, l::3, :], a1s[:, l::3, :], s1b[:, l:l + 1])
            for k in range(NA):
                ms = slice(3 * k, min(3 * k + 3, NCHUNK))
                nc.scalar.mul(t2[:, ms, :], a2s[:, ms, :], s2b[:, k:k + 1])
            nc.vector.tensor_add(
                wc.rearrange("p a b -> p (a b)"),
                t1.rearrange("p a b -> p (a b)"),
                t2.rearrange("p a b -> p (a b)"))

            # ---- main sweep, pipelined over the two column groups ----
            # psum bank b of group g: partitions 0:64 = local block b
            # (true block TB[g][b]), partitions 64:128 = local block b+2
            # (true block TB[g][b]+4); column-tiled matmul pairs.
            for g, gsl in enumerate(gsls):
                banks = [psum.tile([128, NSUB], dt.float32, tag="acc",
                                   name=f"bank{g}_{b}") for b in range(2)]
                for m in range(NCHUNK):
                    k, l = divmod(m, 3)
                    f = featp.tile([128, HALF], dt.bfloat16, tag="f")
                    nc.vector.tensor_mul(f, a_tiles[k][:, gsl],
                                         b_tiles[l][:, gsl])
                    for b in range(2):
                        for u in range(2):
                            cols = slice((2 * u + b) * NSUB,
                                         (2 * u + b + 1) * NSUB)
                            nc.tensor.matmul(
                                banks[b][64 * u:64 * (u + 1), :],
                                wc[:, m, :], f[:, cols],
                                start=(m == 0),
                                stop=(m == NCHUNK - 1),
                                skip_group_check=True)
                    if m == 5:
                        # conv_b term: += B @ x (bf16 identity init -> +x)
                        for b in range(2):
                            for u in range(2):
                                col0 = g * HALF + (2 * u + b) * NSUB
                                nc.tensor.matmul(
                                    banks[b][64 * u:64 * (u + 1), :],
                                    ids, xb[:, col0:col0 + NSUB],
                                    start=False, stop=False,
                                    skip_group_check=True)
                # assemble [128, 1024] staging tile; out rows u hold blocks
                # (2g+4u, 2g+4u+1) = contiguous 1024-pixel run in out
                ot = outsp.tile([128, 2 * NSUB], dt.float32, tag="o")
                for b in range(2):
                    nc.scalar.copy(ot[:, b * NSUB:(b + 1) * NSUB], banks[b])
                for u in range(2):
                    dst = bass.AP(tensor=out_d.ap().tensor,
                                  offset=out_d.ap().offset + g * 1024 + u * 2048,
                                  ap=[[P, C], [NSUB, 2], [1, NSUB]])
                    eng = nc.sync if u == 0 else nc.gpsimd
                    eng.dma_start(out=dst, in_=ot[64 * u:64 * (u + 1), :])

    nc.compile()
    return nc


def _get_program(niter=None):
    key = ("nc", niter)
    if key not in _CACHE:
        _CACHE[key] = _build_program(niter)
    return _CACHE[key]


def kernel(x, fc1_w, fc2_w, conv_w, conv_b):
    from concourse.bass_utils import run_bass_kernel_spmd

    x = np.asarray(x, np.float32)
    a1, a2, fc1t, fc2t = _host_weights(
        np.asarray(conv_w, np.float32), np.asarray(fc1_w, np.float32),
        np.asarray(fc2_w, np.float32))
    # conv_b contributes sum_i B[c,i]*x_i with B = conv_b.reshape(C, C); the
    # "residual" matmul realizes it with lhsT = B.T (identity-init -> +x).
    identb = np.ascontiguousarray(
        np.asarray(conv_b, np.float32).reshape(C, C).T.astype(ml_dtypes.bfloat16))
    nc = _get_program()
    in_maps = []
    for b in range(N_CORES):
        in_maps.append({
            "x": np.ascontiguousarray(x[b].reshape(C, P)),
            "a1": a1, "a2": a2, "fc1t": fc1t, "fc2t": fc2t, "identb": identb,
        })
    res = run_bass_kernel_spmd(nc, in_maps, core_ids=list(range(N_CORES)))
    out = np.stack([res.results[b]["out"].reshape(C, H, W)
                    for b in range(N_CORES)], axis=0)
    return out.astype(np.float32)
